# revision 24
# baseline (speedup 1.0000x reference)
"""Llama-style GQA attention (S=4096, H=2048, 16 q heads / 4 kv heads, d=128, fp32)
on 8 Trainium2 NeuronCores.

Sharding: 4 head-groups x 2 sequence-halves. Core c = 2*g + sh owns q heads
[4g, 4g+4) (one kv head g) and query rows [2048*sh, 2048*(sh+1)). Each core
computes its partial o_proj output transposed ([out_feat, seq]) in bf16; the
host sums the 4 head-group partials per sequence half and concatenates.

v2 layout/schedule (vs v1):
  - single hs stream: Q projection is computed from the same hs chunks as K/V
    (own-half chunks first via a host-side column permutation of hs/cos/sin;
    key order is softmax-invariant), no separate hs_q load
  - 512-wide proj PSUM tiles; PSUM plan: psP(small,2) + psS(scores 1024,2) +
    psAV(2) = 8 banks; chunk 0 split into 4 sub-loads so the first proj
    matmuls start as data lands
  - exp table pre-warmed during projections; denominators via all-ones
    [128,128] broadcast matmul on DVE-folded acc + reciprocal_approx_fast
    (replaces [1,512] serial reciprocal + broadcast matmul, which stalled
    every tile ~3.3us)
  - exp written into halves of a [128,2048] pt tile so denominator
    accumulation adds run 2048 wide on DVE
  - norm chain of tile (t,h) emitted mid-way through tile (t,h+1); o_proj of
    sqt t-1 interleaved into attention of sqt t, so PE fills ACT-paced gaps
  - V-evacuation copies on ScalarE (idle during projections); input DMA split
    across the sync and scalar HWDGE queues; bf16 partial output (host sums
    in fp32)

Measured (8-core axon trn2): ~472-477us vs 633us for v1 (PE-bound: ~419us
matmul streaming at ~97% of the bf16 roofline; ACT exp ~285us overlapped).
Rejected: fp8 anywhere (rel_absmax 0.024-0.08 > 2e-2 tolerance), pair-wise
K/V AllGather dedup (2-rank 1MB collective costs ~100us here), 16-bit PSUM
matmul output (TRN3-only).
"""

import math

import numpy as np
import ml_dtypes

_S, _H, _HD = 4096, 2048, 128
_NCORES = 8
_SQ = _S // 2          # per-core query rows (2048)
_BF16 = ml_dtypes.bfloat16


def _build_nc():
    import concourse.bacc as bacc
    import concourse.mybir as mybir
    import concourse.tile as tile

    dt = mybir.dt
    F32, BF16, F16 = dt.float32, dt.bfloat16, dt.float16
    AF = mybir.ActivationFunctionType

    nc = bacc.Bacc("TRN2", target_bir_lowering=False, debug=False,
                   num_devices=_NCORES)

    def din(name, shape, dtype):
        return nc.dram_tensor(name, shape, dtype, kind="ExternalInput").ap()

    hs_l = din("hs_l", [128, 16 * 4096], BF16)     # hsT h-blocked, full seq
    wq_l = din("wq_l", [128, 16 * 512], BF16)      # wqT h-blocked (pre-scaled)
    wk_l = din("wk_l", [128, 16 * 128], BF16)
    wv_l = din("wv_l", [128, 16 * 128], BF16)
    wo_l = din("wo_l", [128, 4 * 2048], BF16)      # woT hd-blocked
    cos_k = din("cos_k", [128, 4096], F16)
    sinm_k = din("sinm_k", [128, 4096], F16)
    onesb = din("onesb", [128, 128], BF16)         # all-ones (denom broadcast)
    ident = din("ident", [128, 128], BF16)
    outT = nc.dram_tensor("outT", [2048, 2048], BF16, kind="ExternalOutput").ap()

    # One SPMD program for all cores: each core's hs/cos/sin columns are
    # host-permuted so its own sequence half comes first. Keys/values then
    # live in permuted order (softmax and attn@V sums are order-invariant),
    # and chunks 0..3 are always both the K/V-own and the Q chunks.

    with tile.TileContext(nc) as tc:
        with (
            tc.tile_pool(name="wp", bufs=1) as wp,
            tc.tile_pool(name="bigp", bufs=1) as bigp,
            tc.tile_pool(name="hsp", bufs=3) as hsp,
            tc.tile_pool(name="vtt", bufs=2) as vttp,
            tc.tile_pool(name="ptp", bufs=4) as ptp,
            tc.tile_pool(name="accp", bufs=2) as accp,
            tc.tile_pool(name="attnp", bufs=2) as attnp,
            tc.tile_pool(name="outp", bufs=3) as outp,
            tc.tile_pool(name="tmpp", bufs=2) as tmpp,
            tc.tile_pool(name="rbp", bufs=2) as rbp,
            tc.tile_pool(name="psP", bufs=2, space="PSUM") as psP,    # 2 banks
            tc.tile_pool(name="psS", bufs=2, space="PSUM") as psS,    # 4 banks
            tc.tile_pool(name="psAV", bufs=2, space="PSUM") as psAV,  # 2 banks
        ):
            # ---- resident weights/tables.  Emission order = queue priority.
            # sync queue: wk, id, hst0, cosk, hst2, hst4, hst6, onesb, wo
            # scalar queue: wv, wq, sinmk, hst1, hst3, hst5, hst7
            wk_sb = wp.tile([128, 16 * 128], BF16, name="wk_sb")
            nc.sync.dma_start(wk_sb[:, :], wk_l[:, :])
            wv_sb = wp.tile([128, 16 * 128], BF16, name="wv_sb")
            nc.scalar.dma_start(wv_sb[:, :], wv_l[:, :])
            id_sb = wp.tile([128, 128], BF16, name="id_sb")
            wq_sb = wp.tile([128, 16 * 512], BF16, name="wq_sb")
            nc.scalar.dma_start(wq_sb[:, :], wq_l[:, :])
            cosk_sb = wp.tile([128, 4096], F16, name="cosk_sb")
            sinmk_sb = wp.tile([128, 4096], F16, name="sinmk_sb")
            nc.scalar.dma_start(sinmk_sb[:, :], sinm_k[:, :])
            onesb_sb = wp.tile([128, 128], BF16, name="onesb_sb")
            wo_sb = wp.tile([128, 4 * 2048], BF16, name="wo_sb")

            # ---- persistent activations
            qr = bigp.tile([128, 4 * 2048], BF16, name="qr")    # [d, qh*2048+sq]
            kr = bigp.tile([128, 4096], BF16, name="kr")        # [d, sk]
            vsb = bigp.tile([128, 4096], BF16, name="vsb")      # [sk%128, jt*128+d]

            hs3 = hs_l.rearrange("p (t s) -> p t s", t=16)

            def rope(dst, ps, c0):
                # dst = ps * cos + swap_halves(ps) * sinm  (partition dim = d)
                t1 = tmpp.tile([128, 512], F32, name="t1", tag="t1")
                t2 = tmpp.tile([128, 512], F32, name="t2", tag="t2")
                nc.vector.tensor_mul(t1[:, :], ps[:, :], cosk_sb[:, c0:c0 + 512])
                nc.vector.tensor_mul(t2[0:64, :], ps[64:128, :],
                                     sinmk_sb[0:64, c0:c0 + 512])
                nc.vector.tensor_mul(t2[64:128, :], ps[0:64, :],
                                     sinmk_sb[64:128, c0:c0 + 512])
                nc.vector.tensor_add(dst, t1[:, :], t2[:, :])

            # chunk schedule (hs columns are host-permuted so own half = first).
            # Chunk 0 is split into 4 sub-loads so the first proj matmuls can
            # start as the early ht-blocks land; later chunks are prefetched
            # well ahead, and each dma_start costs ~0.7us of sequencer issue
            # time, so they stay single transfers.
            def load_hst(c, engine):
                hst = hsp.tile([128, 16 * 512], BF16, name="hst", tag="hst")
                h3 = hst.rearrange("p (t s) -> p t s", t=16)
                nsub = 4 if c == 0 else 1
                step = 16 // nsub
                for k in range(nsub):
                    engine.dma_start(
                        h3[:, step * k:step * (k + 1), :],
                        hs3[:, step * k:step * (k + 1),
                            c * 512:(c + 1) * 512])
                return hst

            def proj_16(ps, w_sb, hst):
                for ht in range(16):
                    nc.tensor.matmul(
                        ps[:, :],
                        w_sb[:, ht * 128:(ht + 1) * 128],
                        hst[:, ht * 512:(ht + 1) * 512],
                        start=(ht == 0), stop=(ht == 15))

            def kv_chunk(c, hst):
                psk = psP.tile([128, 512], F32, name="psk", tag="psP")
                proj_16(psk, wk_sb, hst)
                rope(kr[:, c * 512:(c + 1) * 512], psk, c * 512)
                psv = psP.tile([128, 512], F32, name="psv", tag="psP")
                proj_16(psv, wv_sb, hst)
                vt = vttp.tile([128, 512], BF16, name="vt", tag="vt")
                nc.scalar.copy(vt[:, :], psv[:, :])
                for j in range(4):  # VT[d, s] -> V[s, d] via PE transpose
                    pst = psP.tile([128, 128], BF16, name="pst", tag="psP")
                    nc.tensor.transpose(pst[:, :], vt[:, j * 128:(j + 1) * 128],
                                        id_sb[:, :])
                    jt = 4 * c + j
                    nc.scalar.copy(vsb[:, jt * 128:(jt + 1) * 128], pst[:, :])

            def q_chunk(c, hst):
                # chunk c < 4 (own half); local query cols = c*512
                for qd in range(4):
                    psq = psP.tile([128, 512], F32, name="psq", tag="psP")
                    for ht in range(16):
                        nc.tensor.matmul(
                            psq[:, :],
                            wq_sb[:, ht * 512 + qd * 128: ht * 512 + (qd + 1) * 128],
                            hst[:, ht * 512:(ht + 1) * 512],
                            start=(ht == 0), stop=(ht == 15))
                    rope(qr[:, qd * 2048 + c * 512: qd * 2048 + (c + 1) * 512],
                         psq, c * 512)

            # ---- projections: own chunks carry K,V and Q; others K,V only
            for ci in range(8):
                eng = nc.sync if ci % 2 == 0 else nc.scalar
                hst = load_hst(ci, eng)
                if ci == 0:
                    nc.sync.dma_start(id_sb[:, :], ident[:, :])
                    nc.sync.dma_start(cosk_sb[:, :], cos_k[:, :])
                if ci == 3:
                    nc.sync.dma_start(onesb_sb[:, :], onesb[:, :])
                    nc.sync.dma_start(wo_sb[:, :], wo_l[:, :])
                kv_chunk(ci, hst)
                if ci < 4:
                    q_chunk(ci, hst)
                if ci == 0:
                    # pre-warm the exp table set while ACT is idle
                    wrm = vttp.tile([128, 16], BF16, name="wrm", tag="wrm")
                    nc.scalar.activation(wrm[:, :], id_sb[:, 0:16], AF.Exp)

            # ---- attention + o_proj, interleaved emission
            at_tiles = {}
            pending = None  # (h_slice_dst, psav, acc)

            def emit_norm(p):
                dst, psav, acc = p
                # fold the two acc halves on DVE so psd needs 2 MMs, not 4
                nc.vector.tensor_add(acc[:, 0:1024], acc[:, 0:1024],
                                     acc[:, 1024:2048])
                psd = psP.tile([128, 512], F32, name="psd", tag="psP")
                for q in range(2):
                    nc.tensor.matmul(psd[:, :], onesb_sb[:, :],
                                     acc[:, q * 512:(q + 1) * 512],
                                     start=(q == 0), stop=(q == 1))
                rb = rbp.tile([128, 512], F32, name="rb", tag="rb")
                nc.vector.reciprocal_approx_fast(rb[:, :], psd[:, :])
                nc.vector.tensor_mul(dst, psav[:, :], rb[:, :])

            def emit_oproj(t, ot):
                at_t = at_tiles[t]
                pso = psP.tile([128, 512], F32, name="pso", tag="psP")
                for hdt in range(4):
                    nc.tensor.matmul(
                        pso[:, :],
                        wo_sb[:, hdt * 2048 + ot * 128: hdt * 2048 + (ot + 1) * 128],
                        at_t[:, hdt * 512:(hdt + 1) * 512],
                        start=(hdt == 0), stop=(hdt == 3))
                osb = outp.tile([128, 512], BF16, name="osb", tag="osb")
                nc.vector.tensor_copy(osb[:, :], pso[:, :])
                nc.sync.dma_start(
                    outT[ot * 128:(ot + 1) * 128, t * 512:(t + 1) * 512],
                    osb[:, :])

            for t in range(4):
                at_tiles[t] = attnp.tile([128, 4 * 512], BF16,
                                         name=f"at{t}", tag="at")
                for h in range(4):
                    qsl = qr[:, h * 2048 + t * 512: h * 2048 + (t + 1) * 512]
                    psav = psAV.tile([128, 512], F32, name="psav", tag="psAV")
                    acc = accp.tile([128, 2048], BF16, name="acc", tag="acc")
                    for idx in range(8):  # 4 key-blocks (512 keys) per step
                        pt = ptp.tile([128, 2048], BF16, name="pt", tag="pt")
                        for half in range(2):
                            pss = psS.tile([128, 1024], F32, name="pss",
                                           tag="psS")
                            for j2 in range(2):
                                jt = 4 * idx + 2 * half + j2
                                nc.tensor.matmul(
                                    pss[:, j2 * 512:(j2 + 1) * 512],
                                    kr[:, jt * 128:(jt + 1) * 128], qsl,
                                    start=True, stop=True)
                            nc.scalar.activation(
                                pt[:, half * 1024:(half + 1) * 1024],
                                pss[:, :], AF.Exp)
                            for j2 in range(2):
                                jt = 4 * idx + 2 * half + j2
                                nc.tensor.matmul(
                                    psav[:, :],
                                    vsb[:, jt * 128:(jt + 1) * 128],
                                    pt[:, (2 * half + j2) * 512:
                                       (2 * half + j2 + 1) * 512],
                                    start=(idx == 0 and half == 0 and j2 == 0),
                                    stop=(idx == 7 and half == 1 and j2 == 1))
                        if idx == 0:
                            nc.vector.tensor_copy(acc[:, :], pt[:, :])
                        else:
                            nc.vector.tensor_add(acc[:, :], acc[:, :], pt[:, :])
                        if idx == 3 and pending is not None:
                            emit_norm(pending)
                            pending = None
                    pending = (at_tiles[t][:, h * 512:(h + 1) * 512], psav, acc)
                    if t > 0:
                        for ot in range(4 * h, 4 * h + 4):
                            emit_oproj(t - 1, ot)
            emit_norm(pending)
            for ot in range(16):
                emit_oproj(3, ot)

    nc.compile()
    return nc


def _blocks_p(x):
    """[(T*128), C] row-major -> [128, T*C] with block t at cols [t*C,(t+1)*C)."""
    t = x.shape[0] // 128
    return np.ascontiguousarray(
        x.reshape(t, 128, -1).transpose(1, 0, 2).reshape(128, -1))


def _prepare_in_maps(hidden_states, wq, wk, wv, wo):
    hs = np.ascontiguousarray(np.asarray(hidden_states, np.float32)[0])  # [S,H]
    hsT = np.ascontiguousarray(hs.T)                                     # [H,S]
    hsT_b = hsT.astype(_BF16)

    inv_freq = 1.0 / (10000.0 ** (np.arange(0, _HD, 2, dtype=np.float32) / _HD))
    t = np.arange(_S, dtype=np.float32)
    freqs = np.einsum("i,j->ij", t, inv_freq)
    emb = np.concatenate([freqs, freqs], axis=-1)                        # [S,128]
    cosT = np.ascontiguousarray(np.cos(emb).T.astype(np.float16))         # [128,S]
    sinm = np.sin(emb).astype(np.float32)
    sinm[:, :64] *= -1.0
    sinmT = np.ascontiguousarray(sinm.T.astype(np.float16))

    scale = 1.0 / math.sqrt(_HD)
    wq = np.asarray(wq, np.float32)
    wk = np.asarray(wk, np.float32)
    wv = np.asarray(wv, np.float32)
    wo = np.asarray(wo, np.float32)

    onesb = np.ones((128, 128), np.float32).astype(_BF16)
    ident = np.eye(128, dtype=np.float32).astype(_BF16)

    in_maps = []
    for c in range(_NCORES):
        g, sh = c // 2, c % 2
        # key-order permutation: own seq half first (order-invariant for
        # softmax/attn sums; queries are never permuted)
        if sh == 0:
            hs_perm = hsT_b
            cos_p, sinm_p = cosT, sinmT
        else:
            hs_perm = np.concatenate(
                [hsT_b[:, _SQ:], hsT_b[:, :_SQ]], axis=1)
            cos_p = np.ascontiguousarray(
                np.concatenate([cosT[:, _SQ:], cosT[:, :_SQ]], axis=1))
            sinm_p = np.ascontiguousarray(
                np.concatenate([sinmT[:, _SQ:], sinmT[:, :_SQ]], axis=1))
        in_maps.append({
            "hs_l": _blocks_p(np.ascontiguousarray(hs_perm)),
            "wq_l": _blocks_p(
                (wq[512 * g:512 * (g + 1), :].T * scale).astype(_BF16)),
            "wk_l": _blocks_p(wk[128 * g:128 * (g + 1), :].T.astype(_BF16)),
            "wv_l": _blocks_p(wv[128 * g:128 * (g + 1), :].T.astype(_BF16)),
            "wo_l": _blocks_p(
                np.ascontiguousarray(wo[:, 512 * g:512 * (g + 1)].T).astype(_BF16)),
            "cos_k": cos_p,
            "sinm_k": sinm_p,
            "onesb": onesb,
            "ident": ident,
        })
    return in_maps


def _run(inputs, trace=False):
    from concourse.bass_utils import run_bass_kernel_spmd

    nc = _build_nc()
    in_maps = _prepare_in_maps(**inputs)
    res = run_bass_kernel_spmd(nc, in_maps, core_ids=list(range(_NCORES)),
                               trace=trace)
    halves = []
    for sh in range(2):
        acc = np.zeros((2048, 2048), np.float32)
        for g in range(4):
            acc += np.asarray(res.results[2 * g + sh]["outT"], np.float32)
        halves.append(acc.T)
    out = np.concatenate(halves, axis=0)[None]
    return np.ascontiguousarray(out, dtype=np.float32), res


def kernel(**inputs):
    out, _ = _run(inputs, trace=False)
    return out


# revision 25
# speedup vs baseline: 1.0057x; 1.0057x over previous
"""Llama-style GQA attention (S=4096, H=2048, 16 q heads / 4 kv heads, d=128, fp32)
on 8 Trainium2 NeuronCores.

Sharding: 4 head-groups x 2 sequence-halves. Core c = 2*g + sh owns q heads
[4g, 4g+4) (one kv head g) and query rows [2048*sh, 2048*(sh+1)). Each core
computes its partial o_proj output transposed ([out_feat, seq]) in bf16; the
host sums the 4 head-group partials per sequence half and concatenates.

v2 layout/schedule (vs v1):
  - single hs stream: Q projection is computed from the same hs chunks as K/V
    (own-half chunks first via a host-side column permutation of hs/cos/sin;
    key order is softmax-invariant), no separate hs_q load
  - 512-wide proj PSUM tiles; PSUM plan: psP(small,2) + psS(scores 1024,2) +
    psAV(2) = 8 banks; chunk 0 split into 4 sub-loads so the first proj
    matmuls start as data lands
  - exp table pre-warmed during projections; denominators via all-ones
    [128,128] broadcast matmul on DVE-folded acc + reciprocal_approx_fast
    (replaces [1,512] serial reciprocal + broadcast matmul, which stalled
    every tile ~3.3us)
  - exp written into halves of a [128,2048] pt tile so denominator
    accumulation adds run 2048 wide on DVE
  - norm chain of tile (t,h) emitted mid-way through tile (t,h+1); o_proj of
    sqt t-1 interleaved into attention of sqt t, so PE fills ACT-paced gaps
  - V-evacuation copies on ScalarE (idle during projections); input DMA split
    across the sync and scalar HWDGE queues; bf16 partial output (host sums
    in fp32)

Measured (8-core axon trn2): ~472-477us vs 633us for v1 (PE-bound: ~419us
matmul streaming at ~97% of the bf16 roofline; ACT exp ~285us overlapped).
Rejected: fp8 anywhere (rel_absmax 0.024-0.08 > 2e-2 tolerance), pair-wise
K/V AllGather dedup (2-rank 1MB collective costs ~100us here), 16-bit PSUM
matmul output (TRN3-only).
"""

import math

import numpy as np
import ml_dtypes

_S, _H, _HD = 4096, 2048, 128
_NCORES = 8
_SQ = _S // 2          # per-core query rows (2048)
_BF16 = ml_dtypes.bfloat16


def _build_nc():
    import concourse.bacc as bacc
    import concourse.mybir as mybir
    import concourse.tile as tile

    dt = mybir.dt
    F32, BF16, F16 = dt.float32, dt.bfloat16, dt.float16
    AF = mybir.ActivationFunctionType

    nc = bacc.Bacc("TRN2", target_bir_lowering=False, debug=False,
                   num_devices=_NCORES)

    def din(name, shape, dtype):
        return nc.dram_tensor(name, shape, dtype, kind="ExternalInput").ap()

    hs_l = din("hs_l", [128, 16 * 4096], BF16)     # hsT h-blocked, full seq
    wq_l = din("wq_l", [128, 16 * 512], BF16)      # wqT h-blocked (pre-scaled)
    wk_l = din("wk_l", [128, 16 * 128], BF16)
    wv_l = din("wv_l", [128, 16 * 128], BF16)
    wo_l = din("wo_l", [128, 4 * 2048], BF16)      # woT hd-blocked
    cos_k = din("cos_k", [128, 4096], F16)
    sinm_k = din("sinm_k", [128, 4096], F16)
    onesb = din("onesb", [128, 128], BF16)         # all-ones (denom broadcast)
    ident = din("ident", [128, 128], BF16)
    outT = nc.dram_tensor("outT", [2048, 2048], BF16, kind="ExternalOutput").ap()

    # One SPMD program for all cores: each core's hs/cos/sin columns are
    # host-permuted so its own sequence half comes first. Keys/values then
    # live in permuted order (softmax and attn@V sums are order-invariant),
    # and chunks 0..3 are always both the K/V-own and the Q chunks.

    with tile.TileContext(nc) as tc:
        with (
            tc.tile_pool(name="wp", bufs=1) as wp,
            tc.tile_pool(name="bigp", bufs=1) as bigp,
            tc.tile_pool(name="hsp", bufs=3) as hsp,
            tc.tile_pool(name="vtt", bufs=2) as vttp,
            tc.tile_pool(name="ptp", bufs=3) as ptp,
            tc.tile_pool(name="accp", bufs=2) as accp,
            tc.tile_pool(name="attnp", bufs=2) as attnp,
            tc.tile_pool(name="outp", bufs=3) as outp,
            tc.tile_pool(name="tmpp", bufs=2) as tmpp,
            tc.tile_pool(name="rbp", bufs=2) as rbp,
            tc.tile_pool(name="psP", bufs=2, space="PSUM") as psP,    # 2 banks
            tc.tile_pool(name="psS", bufs=2, space="PSUM") as psS,    # 4 banks
            tc.tile_pool(name="psAV", bufs=2, space="PSUM") as psAV,  # 2 banks
        ):
            # ---- resident weights/tables.  Emission order = queue priority.
            # sync queue: wk, id, hst0, cosk, hst2, hst4, hst6, onesb, wo
            # scalar queue: wv, wq, sinmk, hst1, hst3, hst5, hst7
            wk_sb = wp.tile([128, 16 * 128], BF16, name="wk_sb")
            nc.sync.dma_start(wk_sb[:, :], wk_l[:, :])
            wv_sb = wp.tile([128, 16 * 128], BF16, name="wv_sb")
            nc.scalar.dma_start(wv_sb[:, :], wv_l[:, :])
            id_sb = wp.tile([128, 128], BF16, name="id_sb")
            wq_sb = wp.tile([128, 16 * 512], BF16, name="wq_sb")
            nc.scalar.dma_start(wq_sb[:, :], wq_l[:, :])
            cosk_sb = wp.tile([128, 4096], F16, name="cosk_sb")
            sinmk_sb = wp.tile([128, 4096], F16, name="sinmk_sb")
            nc.scalar.dma_start(sinmk_sb[:, :], sinm_k[:, :])
            onesb_sb = wp.tile([128, 128], BF16, name="onesb_sb")
            wo_sb = wp.tile([128, 4 * 2048], BF16, name="wo_sb")

            # ---- persistent activations
            qr = bigp.tile([128, 4 * 2048], BF16, name="qr")    # [d, qh*2048+sq]
            kr = bigp.tile([128, 4096], BF16, name="kr")        # [d, sk]
            vsb = bigp.tile([128, 4096], BF16, name="vsb")      # [sk%128, jt*128+d]

            hs3 = hs_l.rearrange("p (t s) -> p t s", t=16)

            def rope(dst, ps, c0):
                # dst = ps * cos + swap_halves(ps) * sinm  (partition dim = d)
                t1 = tmpp.tile([128, 512], F32, name="t1", tag="t1")
                t2 = tmpp.tile([128, 512], F32, name="t2", tag="t2")
                nc.vector.tensor_mul(t1[:, :], ps[:, :], cosk_sb[:, c0:c0 + 512])
                nc.vector.tensor_mul(t2[0:64, :], ps[64:128, :],
                                     sinmk_sb[0:64, c0:c0 + 512])
                nc.vector.tensor_mul(t2[64:128, :], ps[0:64, :],
                                     sinmk_sb[64:128, c0:c0 + 512])
                nc.vector.tensor_add(dst, t1[:, :], t2[:, :])

            # chunk schedule (hs columns are host-permuted so own half = first).
            # Chunk 0 is split into 4 sub-loads so the first proj matmuls can
            # start as the early ht-blocks land; later chunks are prefetched
            # well ahead, and each dma_start costs ~0.7us of sequencer issue
            # time, so they stay single transfers.
            def load_hst(c, engine):
                hst = hsp.tile([128, 16 * 512], BF16, name="hst", tag="hst")
                h3 = hst.rearrange("p (t s) -> p t s", t=16)
                nsub = 4 if c == 0 else 1
                step = 16 // nsub
                for k in range(nsub):
                    engine.dma_start(
                        h3[:, step * k:step * (k + 1), :],
                        hs3[:, step * k:step * (k + 1),
                            c * 512:(c + 1) * 512])
                return hst

            def proj_16(ps, w_sb, hst):
                for ht in range(16):
                    nc.tensor.matmul(
                        ps[:, :],
                        w_sb[:, ht * 128:(ht + 1) * 128],
                        hst[:, ht * 512:(ht + 1) * 512],
                        start=(ht == 0), stop=(ht == 15))

            def kv_chunk(c, hst):
                psk = psP.tile([128, 512], F32, name="psk", tag="psP")
                proj_16(psk, wk_sb, hst)
                rope(kr[:, c * 512:(c + 1) * 512], psk, c * 512)
                psv = psP.tile([128, 512], F32, name="psv", tag="psP")
                proj_16(psv, wv_sb, hst)
                vt = vttp.tile([128, 512], BF16, name="vt", tag="vt")
                nc.scalar.copy(vt[:, :], psv[:, :])
                for j in range(4):  # VT[d, s] -> V[s, d] via PE transpose
                    pst = psP.tile([128, 128], BF16, name="pst", tag="psP")
                    nc.tensor.transpose(pst[:, :], vt[:, j * 128:(j + 1) * 128],
                                        id_sb[:, :])
                    jt = 4 * c + j
                    nc.scalar.copy(vsb[:, jt * 128:(jt + 1) * 128], pst[:, :])

            def q_chunk(c, hst):
                # chunk c < 4 (own half); local query cols = c*512
                for qd in range(4):
                    psq = psP.tile([128, 512], F32, name="psq", tag="psP")
                    for ht in range(16):
                        nc.tensor.matmul(
                            psq[:, :],
                            wq_sb[:, ht * 512 + qd * 128: ht * 512 + (qd + 1) * 128],
                            hst[:, ht * 512:(ht + 1) * 512],
                            start=(ht == 0), stop=(ht == 15))
                    rope(qr[:, qd * 2048 + c * 512: qd * 2048 + (c + 1) * 512],
                         psq, c * 512)

            # ---- projections: own chunks carry K,V and Q; others K,V only
            for ci in range(8):
                eng = nc.sync if ci % 2 == 0 else nc.scalar
                hst = load_hst(ci, eng)
                if ci == 0:
                    nc.sync.dma_start(id_sb[:, :], ident[:, :])
                    nc.sync.dma_start(cosk_sb[:, :], cos_k[:, :])
                if ci == 3:
                    nc.sync.dma_start(onesb_sb[:, :], onesb[:, :])
                    nc.sync.dma_start(wo_sb[:, :], wo_l[:, :])
                kv_chunk(ci, hst)
                if ci < 4:
                    q_chunk(ci, hst)
                if ci == 0:
                    # pre-warm the exp table set while ACT is idle
                    wrm = vttp.tile([128, 16], BF16, name="wrm", tag="wrm")
                    nc.scalar.activation(wrm[:, :], id_sb[:, 0:16], AF.Exp)

            # ---- attention + o_proj, interleaved emission
            at_tiles = {}
            pending = None  # (h_slice_dst, psav, acc)

            def emit_norm(p):
                dst, psav, acc = p
                # fold the two acc halves on DVE so psd needs 2 MMs, not 4
                nc.vector.tensor_add(acc[:, 0:1024], acc[:, 0:1024],
                                     acc[:, 1024:2048])
                psd = psP.tile([128, 512], F32, name="psd", tag="psP")
                for q in range(2):
                    nc.tensor.matmul(psd[:, :], onesb_sb[:, :],
                                     acc[:, q * 512:(q + 1) * 512],
                                     start=(q == 0), stop=(q == 1))
                rb = rbp.tile([128, 512], F32, name="rb", tag="rb")
                nc.vector.reciprocal_approx_fast(rb[:, :], psd[:, :])
                nc.vector.tensor_mul(dst, psav[:, :], rb[:, :])

            def emit_oproj(t, ot):
                at_t = at_tiles[t]
                pso = psP.tile([128, 512], F32, name="pso", tag="psP")
                for hdt in range(4):
                    nc.tensor.matmul(
                        pso[:, :],
                        wo_sb[:, hdt * 2048 + ot * 128: hdt * 2048 + (ot + 1) * 128],
                        at_t[:, hdt * 512:(hdt + 1) * 512],
                        start=(hdt == 0), stop=(hdt == 3))
                osb = outp.tile([128, 512], BF16, name="osb", tag="osb")
                nc.vector.tensor_copy(osb[:, :], pso[:, :])
                nc.sync.dma_start(
                    outT[ot * 128:(ot + 1) * 128, t * 512:(t + 1) * 512],
                    osb[:, :])

            for t in range(4):
                at_tiles[t] = attnp.tile([128, 4 * 512], BF16,
                                         name=f"at{t}", tag="at")
                for h in range(4):
                    qsl = qr[:, h * 2048 + t * 512: h * 2048 + (t + 1) * 512]
                    psav = psAV.tile([128, 512], F32, name="psav", tag="psAV")
                    acc = accp.tile([128, 2048], BF16, name="acc", tag="acc")
                    for idx in range(8):  # 4 key-blocks (512 keys) per step
                        pt = ptp.tile([128, 2048], BF16, name="pt", tag="pt")
                        for half in range(2):
                            pss = psS.tile([128, 1024], F32, name="pss",
                                           tag="psS")
                            for j2 in range(2):
                                jt = 4 * idx + 2 * half + j2
                                nc.tensor.matmul(
                                    pss[:, j2 * 512:(j2 + 1) * 512],
                                    kr[:, jt * 128:(jt + 1) * 128], qsl,
                                    start=True, stop=True)
                            nc.scalar.activation(
                                pt[:, half * 1024:(half + 1) * 1024],
                                pss[:, :], AF.Exp)
                            for j2 in range(2):
                                jt = 4 * idx + 2 * half + j2
                                nc.tensor.matmul(
                                    psav[:, :],
                                    vsb[:, jt * 128:(jt + 1) * 128],
                                    pt[:, (2 * half + j2) * 512:
                                       (2 * half + j2 + 1) * 512],
                                    start=(idx == 0 and half == 0 and j2 == 0),
                                    stop=(idx == 7 and half == 1 and j2 == 1))
                        if idx == 0:
                            nc.vector.tensor_copy(acc[:, :], pt[:, :])
                        else:
                            nc.vector.tensor_add(acc[:, :], acc[:, :], pt[:, :])
                        if idx == 3 and pending is not None:
                            emit_norm(pending)
                            pending = None
                    pending = (at_tiles[t][:, h * 512:(h + 1) * 512], psav, acc)
                    if t > 0:
                        for ot in range(4 * h, 4 * h + 4):
                            emit_oproj(t - 1, ot)
            emit_norm(pending)
            for ot in range(16):
                emit_oproj(3, ot)

    nc.compile()
    return nc


def _blocks_p(x):
    """[(T*128), C] row-major -> [128, T*C] with block t at cols [t*C,(t+1)*C)."""
    t = x.shape[0] // 128
    return np.ascontiguousarray(
        x.reshape(t, 128, -1).transpose(1, 0, 2).reshape(128, -1))


def _prepare_in_maps(hidden_states, wq, wk, wv, wo):
    hs = np.ascontiguousarray(np.asarray(hidden_states, np.float32)[0])  # [S,H]
    hsT = np.ascontiguousarray(hs.T)                                     # [H,S]
    hsT_b = hsT.astype(_BF16)

    inv_freq = 1.0 / (10000.0 ** (np.arange(0, _HD, 2, dtype=np.float32) / _HD))
    t = np.arange(_S, dtype=np.float32)
    freqs = np.einsum("i,j->ij", t, inv_freq)
    emb = np.concatenate([freqs, freqs], axis=-1)                        # [S,128]
    cosT = np.ascontiguousarray(np.cos(emb).T.astype(np.float16))         # [128,S]
    sinm = np.sin(emb).astype(np.float32)
    sinm[:, :64] *= -1.0
    sinmT = np.ascontiguousarray(sinm.T.astype(np.float16))

    scale = 1.0 / math.sqrt(_HD)
    wq = np.asarray(wq, np.float32)
    wk = np.asarray(wk, np.float32)
    wv = np.asarray(wv, np.float32)
    wo = np.asarray(wo, np.float32)

    onesb = np.ones((128, 128), np.float32).astype(_BF16)
    ident = np.eye(128, dtype=np.float32).astype(_BF16)

    in_maps = []
    for c in range(_NCORES):
        g, sh = c // 2, c % 2
        # key-order permutation: own seq half first (order-invariant for
        # softmax/attn sums; queries are never permuted)
        if sh == 0:
            hs_perm = hsT_b
            cos_p, sinm_p = cosT, sinmT
        else:
            hs_perm = np.concatenate(
                [hsT_b[:, _SQ:], hsT_b[:, :_SQ]], axis=1)
            cos_p = np.ascontiguousarray(
                np.concatenate([cosT[:, _SQ:], cosT[:, :_SQ]], axis=1))
            sinm_p = np.ascontiguousarray(
                np.concatenate([sinmT[:, _SQ:], sinmT[:, :_SQ]], axis=1))
        in_maps.append({
            "hs_l": _blocks_p(np.ascontiguousarray(hs_perm)),
            "wq_l": _blocks_p(
                (wq[512 * g:512 * (g + 1), :].T * scale).astype(_BF16)),
            "wk_l": _blocks_p(wk[128 * g:128 * (g + 1), :].T.astype(_BF16)),
            "wv_l": _blocks_p(wv[128 * g:128 * (g + 1), :].T.astype(_BF16)),
            "wo_l": _blocks_p(
                np.ascontiguousarray(wo[:, 512 * g:512 * (g + 1)].T).astype(_BF16)),
            "cos_k": cos_p,
            "sinm_k": sinm_p,
            "onesb": onesb,
            "ident": ident,
        })
    return in_maps


def _run(inputs, trace=False):
    from concourse.bass_utils import run_bass_kernel_spmd

    nc = _build_nc()
    in_maps = _prepare_in_maps(**inputs)
    res = run_bass_kernel_spmd(nc, in_maps, core_ids=list(range(_NCORES)),
                               trace=trace)
    halves = []
    for sh in range(2):
        acc = np.zeros((2048, 2048), np.float32)
        for g in range(4):
            acc += np.asarray(res.results[2 * g + sh]["outT"], np.float32)
        halves.append(acc.T)
    out = np.concatenate(halves, axis=0)[None]
    return np.ascontiguousarray(out, dtype=np.float32), res


def kernel(**inputs):
    out, _ = _run(inputs, trace=False)
    return out
